# revision 32
# baseline (speedup 1.0000x reference)
"""Trainium2 Bass kernel for nn_AttentionSampler.

reference:  energies = sites @ w_site + (local . w_local) + b ; softmax(energies)
Softmax is invariant to the additive constant, so only sites @ attn_w[D:2D]
matters.  Energies are ~N(0, 0.41^2) so the max-subtraction is skipped
(exp stays well inside fp32 range); softmax = exp(e) / sum(exp(e)).

Sharding: sites split along N across 8 cores (62500 rows each, padded to
62592 = 128*489 with zero rows).

Precision: the dot products run in bf16 (rel err ~2e-3, well inside the
2e-2 gate), so sites are cast to bf16 on the host and the device streams
16-bit data -- HBM read traffic is 32 MB/core (the roofline, ~92us at
~350 GB/s) instead of 64.

Layout (host-prepped): the shard is block-transposed to sitesT[k, j, h, p] =
sites[p*489 + j, h*128 + k] so the dot product's contraction dim (d) lands on
SBUF partitions.  Each site-block j is two [128d x 128site] stationary tiles
for the PE array; rhs = w-half [128, 1].  Two accumulating matmuls per block
write energies[:, j] into a single PSUM bank [128, 489].  LDWEIGHTS+MATMUL
pairs pipeline at ~53ns/block, so the Tensor engine (~26us busy) chases the
stream with 3.5x headroom (the DVE dot-product version of this kernel was
210us busy and was the critical path).

Streaming: first 32 blocks go through the two HWDGE rings (sync/scalar
engines, start ~7.5us after the engine barrier); the rest streams through
SWDGE queues 0-3 whose descriptor generation starts when the gpsimd Q7
preamble ends ~8.6us.  Tail chunks shrink so the last blocks land (and
matmul) with minimal exposure.

Collective: ncfw boots ~73us into the NEFF regardless of trigger time, so a
dummy warmup AllGather is triggered first (non-blocking doorbell); it
executes during the stream.  The per-partition exp-sums go out in TWO real
AllGathers: stage A (energies of blocks < SPLIT_J, in their own PSUM tile)
exps and launches while the stream tail is still landing, so its ~20us of
latency-bound ring hops overlap the stream; the tiny stage-B AllGather
queued behind it is chased by ncfw at chunk granularity and completes a few
us after it.  Stage A's gather-load + reduce also run off the critical
path.  Each [128, 1] pack is DVE-transposed first so its DMA is 4
contiguous 128B descriptors instead of 128 4-byte ones (~5us cheaper).
S = sum of all 2048 partials minus 8*92 pad terms (pad sites have energy 0,
exp = 1.0 exactly); out = exp(e) * (1/S).
"""

import os
import sys

if "/opt/trn_rl_repo" not in sys.path:
    sys.path.insert(0, "/opt/trn_rl_repo")

import numpy as np

D = 256
N = 500000
N_CORES = 8
SHARD = N // N_CORES          # 62500 sites per core
MROW = 128                    # sites per block (PSUM partition dim)
J = 489                       # blocks per core (128*489 = 62592 >= 62500)
SHARD_PAD = MROW * J          # 62592
NPAD = SHARD_PAD - SHARD      # 92 zero-pad sites (partition 127, cols 397..488)
CORR = float(NPAD * N_CORES)  # exp(0)=1 contribution of all pads to the sum

HEAD_C = 8                    # blocks per HWDGE head chunk
N_HEAD = 6                    # head chunks: cover HBM until SWDGE descgen
                              # starts at ~8.6us and its queues ramp
HEAD_BLOCKS = HEAD_C * N_HEAD
BODY_SIZES = [16] * 26 + [8, 8, 9]  # 441 blocks via SWDGE; small tail
                              # chunks land (and matmul) with less exposure
assert HEAD_BLOCKS + sum(BODY_SIZES) == J
SPLIT_J = 384                 # energies split: blocks [0, SPLIT_J) get their
                              # exp + AllGather launched while the stream
                              # tail is still landing; the final AllGather
                              # over the last blocks chases it through ncfw
NQ = 4                        # SWDGE queues to rotate over
SINGLE_PACKET = True

_nc_cache = None


def build_nc():
    from concourse import bacc, mybir, tile

    f32 = mybir.dt.float32
    bf16 = mybir.dt.bfloat16
    nc = bacc.Bacc(
        "TRN2",
        target_bir_lowering=False,
        debug=False,
        enable_asserts=False,
        num_devices=N_CORES,
        num_swdge_queues=NQ,
    )
    sitesT = nc.dram_tensor("sitesT", [MROW, J * D], bf16, kind="ExternalInput")
    # w pairs (w_lo[k], w_hi[k]) replicated 128x on the host so the HWDGE load
    # fans out over all 16 SDMA engines (tiny loads increment their completion
    # semaphore by fewer than the 16 the consumer waits for).
    w_rep = nc.dram_tensor("w_rep", [128, D], f32, kind="ExternalInput")
    out = nc.dram_tensor("out", [SHARD_PAD], f32, kind="ExternalOutput")
    # Collective buffers: 512B per rank keeps every rank's shard in the
    # AllGather output 32B-aligned (smaller payloads corrupt on HW).
    cc_in = nc.dram_tensor("cc_in", [128], f32)
    cc_out = nc.dram_tensor("cc_out", [128 * N_CORES], f32, addr_space="Shared")
    cc_wi = nc.dram_tensor("cc_wi", [128], f32)
    cc_wo = nc.dram_tensor("cc_wo", [128 * N_CORES], f32, addr_space="Shared")
    cc_w2i = nc.dram_tensor("cc_w2i", [128], f32)
    cc_w2o = nc.dram_tensor("cc_w2o", [128 * N_CORES], f32, addr_space="Shared")

    out_r = out.ap().rearrange("(p j) -> p j", p=MROW)      # [128, 489]
    sT = sitesT.ap()                                        # [128, 489*256]

    AF = mybir.ActivationFunctionType
    ALU = mybir.AluOpType
    AX = mybir.AxisListType

    with tile.TileContext(nc) as tc:
        with (
            tc.tile_pool(name="head", bufs=4) as head,
            tc.tile_pool(name="body", bufs=8) as body,
            tc.tile_pool(name="consts", bufs=1) as consts,
            tc.tile_pool(name="small", bufs=1) as small,
            tc.tile_pool(name="psum", bufs=1, space="PSUM") as psum,
        ):
            # --- warmup collective: non-blocking doorbell, triggered first.
            # ncfw boots ~73us into the NEFF and runs this during the stream;
            # the gathered bytes are discarded.
            nc.gpsimd.collective_compute(
                "AllGather", ALU.bypass,
                replica_groups=[list(range(N_CORES))],
                ins=[cc_wi.ap().rearrange("(p one) -> p one", one=1)],
                outs=[cc_wo.ap().rearrange("(p one) -> p one", one=1)],
            )

            # --- w: HWDGE f32 load + DVE cast to bf16 [128, 2]
            w_f = consts.tile([128, D], f32)
            nc.sync.dma_start(w_f[:], w_rep.ap())
            w_bf = consts.tile([128, 2], bf16)
            nc.vector.tensor_copy(w_bf[:], w_f[:, 0:2])

            e_psA = psum.tile([MROW, SPLIT_J], f32)
            e_psB = psum.tile([MROW, J - SPLIT_J], f32)
            blockmap = {}

            # --- head: bf16 via the two HWDGE rings
            for c in range(N_HEAD):
                j0 = c * HEAD_C
                tb = head.tile([128, HEAD_C * D], bf16, tag="hd")
                eng = nc.sync if c % 2 == 0 else nc.scalar
                eng.dma_start(tb[:], sT[:, j0 * D:(j0 + HEAD_C) * D])
                for jj in range(HEAD_C):
                    blockmap[j0 + jj] = (tb, jj)

            # --- body: bf16 via SWDGE queues 0..NQ-1 (adding the HWDGE
            # rings as extra body queues was tried and skews per-queue
            # completion enough to expose a long matmul tail)
            j0 = HEAD_BLOCKS
            for c, ch in enumerate(BODY_SIZES):
                t = body.tile([128, ch * D], bf16, tag="chunk")
                inst = nc.gpsimd.dma_start(
                    t[:], sT[:, j0 * D:(j0 + ch) * D], single_packet=SINGLE_PACKET
                )
                qn = c % (NQ - 1)  # queue 3 is reserved for the pack
                if qn:             # DMAs so they never queue behind body
                    inst.ins.queue = f"qPoolDynamic{qn}"  # packets
                for jj in range(ch):
                    blockmap[j0 + jj] = (t, jj)
                j0 += ch

            # --- energies: two accumulating matmuls per block.
            # lhsT = [128 d, 128 sites] stationary, rhs = w half [128, 1].
            for j in range(J):
                t, jj = blockmap[j]
                lo = t[:, jj * D:jj * D + 128]
                hi = t[:, jj * D + 128:(jj + 1) * D]
                dst = (e_psA[:, j:j + 1] if j < SPLIT_J
                       else e_psB[:, j - SPLIT_J:j - SPLIT_J + 1])
                nc.tensor.matmul(dst, lo, w_bf[:, 0:1], start=True, stop=False)
                nc.tensor.matmul(dst, hi, w_bf[:, 1:2], start=False, stop=True)

            # --- exp in two stages; each stage's [128] per-partition sums
            # go out through their own AllGather.  Stage A (blocks < SPLIT_J)
            # fires while the stream tail is still landing, so its ~20us of
            # latency-bound ring hops overlap the stream; the tiny stage-B
            # AllGather queued behind it is chased by ncfw and completes a
            # few us later.  The [128, 1] packs are DVE-transposed first so
            # each DMA is 4 contiguous 128B descriptors instead of 128
            # 4-byte ones.
            outv = consts.tile([MROW, J], f32, name="outv")
            packA = small.tile([128, 32], f32)
            packB = small.tile([128, 32], f32)
            nc.scalar.activation(
                outv[:, 0:SPLIT_J], e_psA[:], AF.Exp,
                bias=0.0, scale=1.0, accum_out=packA[0:MROW, 0:1],
            )
            packA_t = small.tile([128, 32], f32)
            nc.vector.transpose(packA_t[:], packA[:])
            pA = nc.gpsimd.dma_start(
                cc_in.ap().rearrange("(a b) -> a b", b=32),
                packA_t[:].rearrange("(a b) c -> a b c", b=32)[:, 0, :],
            )
            pA.ins.queue = f"qPoolDynamic{NQ - 1}"
            nc.gpsimd.collective_compute(
                "AllGather", ALU.bypass,
                replica_groups=[list(range(N_CORES))],
                ins=[cc_in.ap().rearrange("(p one) -> p one", one=1)],
                outs=[cc_out.ap().rearrange("(p one) -> p one", one=1)],
            )

            nc.scalar.activation(
                outv[:, SPLIT_J:J], e_psB[:], AF.Exp,
                bias=0.0, scale=1.0, accum_out=packB[0:MROW, 0:1],
            )
            packB_t = small.tile([128, 32], f32)
            nc.vector.transpose(packB_t[:], packB[:])
            pB = nc.gpsimd.dma_start(
                cc_w2i.ap().rearrange("(a b) -> a b", b=32),
                packB_t[:].rearrange("(a b) c -> a b c", b=32)[:, 0, :],
            )
            pB.ins.queue = f"qPoolDynamic{NQ - 1}"
            nc.gpsimd.collective_compute(
                "AllGather", ALU.bypass,
                replica_groups=[list(range(N_CORES))],
                ins=[cc_w2i.ap().rearrange("(p one) -> p one", one=1)],
                outs=[cc_w2o.ap().rearrange("(p one) -> p one", one=1)],
            )

            # Stage-A gather lands mid-stream: its load + reduce run off the
            # critical path.  Only the stage-B gather is exposed.
            gtA = small.tile([1, 128 * N_CORES], f32)
            nc.sync.dma_start(gtA[0:1, :], cc_out.ap()[:])
            SA = small.tile([1, 1], f32)
            nc.vector.tensor_reduce(SA[:], gtA[:], axis=AX.X, op=ALU.add)
            gtB = small.tile([1, 128 * N_CORES], f32)
            nc.sync.dma_start(gtB[0:1, :], cc_w2o.ap()[:])
            SB = small.tile([1, 1], f32)
            nc.vector.tensor_reduce(SB[:], gtB[:], axis=AX.X, op=ALU.add)

            # S = all 2048 partials minus the pad contribution;
            # broadcast 1/S to all partitions
            S = small.tile([1, 1], f32)
            nc.vector.tensor_add(S[:], SA[:], SB[:])
            Sc = small.tile([1, 1], f32)
            nc.vector.tensor_scalar_add(Sc[:], S[:], -CORR)
            invS = small.tile([1, 1], f32)
            nc.vector.reciprocal(invS[:], Sc[:])
            invS_b = small.tile([128, 1], f32)
            nc.gpsimd.partition_broadcast(
                invS_b[:], invS[0:1, :], channels=128
            )

            # --- out = exp(e) * (1/S); pad cols of partition 127 carry
            # garbage but are sliced off on the host.
            nc.vector.tensor_scalar_mul(outv[:], outv[:], invS_b[0:MROW, :])
            nc.sync.dma_start(out_r, outv[:])

    nc.compile()
    return nc


def _get_nc():
    global _nc_cache
    if _nc_cache is None:
        _nc_cache = build_nc()
    return _nc_cache


def _prep_shard(shard):
    """[62500, 256] f32 -> bf16 [128, 489*256] with
    sitesT[k, j*256 + h*128 + p] = shard_pad[p*489 + j, h*128 + k]
    (blocked to keep the transpose in-cache).
    """
    import ml_dtypes

    pad = np.zeros((SHARD_PAD, D), ml_dtypes.bfloat16)
    pad[:SHARD] = shard.astype(ml_dtypes.bfloat16)
    padr = pad.reshape(MROW, J, 2, 128)                    # p j h k
    outp = np.empty((128, J, 2, 128), ml_dtypes.bfloat16)  # k j h p
    B = 24
    for b0 in range(0, J, B):
        blk = padr[:, b0:b0 + B]                           # [128, b, 2, 128]
        outp[:, b0:b0 + B] = blk.transpose(3, 1, 2, 0)
    return outp.reshape(128, J * D)


def make_in_maps(sites, attn_w):
    sites = np.ascontiguousarray(np.asarray(sites, dtype=np.float32))
    w_site = np.asarray(attn_w, dtype=np.float32)[D:2 * D]
    pairs = np.empty((128, 2), np.float32)
    pairs[:, 0] = w_site[:128]
    pairs[:, 1] = w_site[128:]
    w_rep = np.ascontiguousarray(np.tile(pairs, (1, D // 2)))
    return [
        {"sitesT": _prep_shard(sites[c * SHARD:(c + 1) * SHARD]), "w_rep": w_rep}
        for c in range(N_CORES)
    ]


def kernel(local, sites, attn_w, attn_b):
    from concourse.bass_utils import run_bass_kernel_spmd

    nc = _get_nc()
    in_maps = make_in_maps(sites, attn_w)
    res = run_bass_kernel_spmd(nc, in_maps, list(range(N_CORES)))
    return np.concatenate(
        [np.asarray(res.results[c]["out"], dtype=np.float32)[:SHARD]
         for c in range(N_CORES)]
    )
